# revision 6
# baseline (speedup 1.0000x reference)
"""Multi-head attention (B=2, T=2048, E=1024, H=16, D=64, RoPE, causal)
on 8 Trainium2 NeuronCores.

Sharding: core c handles batch b=c//4 and head group hg=c%4 (heads
4*hg..4*hg+3).  Each core computes its 4 heads' attention plus its slice
of the output projection; the host sums the 4 partial projections per
batch element.

v6: bf16 x over the wire (Pool upconvert), score scale folded into wq on host, rope epilogue on Pool. v4: ACT carries only the softmax exp (phase-3 psum->sbuf copies all on DVE). v2 changes vs the 64us baseline (HW facts: f32r matmuls stream ~4
rows/cycle ONLY at N=512 - a N=256 matmul costs 2.3x a N=512 one):
  - every score/AV matmul runs the full N=512 moving width; causal
    masking relies on pre-zeroed diagonal prob buffers (left region
    zeroed once, exp never writes it) plus a 128-wide affine_select
    strip on the jagged diagonal only.
  - the softmax denominator comes from a col-tiled all-ones matmul
    pair that lands the denominator replicated across all 128 psum
    partitions - the reciprocal is a single full-width DVE op and the
    DRAM broadcast bounce of the baseline disappears.
  - the AV matmuls are col-tiled (head pairs on array column halves),
    running ~2x, paying back the denominator matmuls.
  - rope epilogue (second multiply + add) moved to the otherwise-idle
    gpsimd/Pool engine; v/y psum->sbuf copies balanced DVE/ACT.
"""

import sys
import os

sys.path.insert(0, "/opt/trn_rl_repo")

import numpy as np
import ml_dtypes

import concourse.bass as bass
import concourse.mybir as mybir
from concourse import bacc, tile
from concourse.bass_utils import run_bass_kernel_spmd

F32 = mybir.dt.float32
BF16 = mybir.dt.bfloat16
F32R = mybir.dt.float32r
AF = mybir.ActivationFunctionType
ALU = mybir.AluOpType

B, T, E = 2, 2048, 1024
H, D = 16, 64
HG = 4            # heads per core
N_CORES = 8
TB = T // 128     # 16 query/key blocks of 128
NCH = T // 512    # 4 query chunks of 512
KC = E // 128     # 8 contraction chunks for the projections

_CACHE = {}


def r(ap):
    return ap.bitcast(F32R)


def build_program(reps=1, phases=(1, 2, 3), serial=False):
    nc = bacc.Bacc("TRN2", num_devices=N_CORES)

    xT_d = nc.declare_dram_parameter("xT", [E, T], BF16, isOutput=False)
    wqk_d = nc.declare_dram_parameter("wqk", [E, 512], F32, isOutput=False)
    wv_d = nc.declare_dram_parameter("wv", [E, 256], F32, isOutput=False)
    wout_d = nc.declare_dram_parameter("wout", [256, E], F32, isOutput=False)
    cosk_d = nc.declare_dram_parameter("cosk", [128, T], F32, isOutput=False)
    sink_d = nc.declare_dram_parameter("sink", [128, T], F32, isOutput=False)
    y_d = nc.declare_dram_parameter("y", [T, E], F32, isOutput=True)

    swap_mask = [i ^ 1 for i in range(32)]  # pairwise swap within 32 lanes

    with tile.TileContext(nc) as tc:
      for _rep in range(reps):
        if serial and _rep > 0:
            tc.strict_bb_all_engine_barrier()
        # ---- persistent pools (live across phases) ----
        persist = tc.alloc_tile_pool(name="persist", bufs=1)
        qkT = [persist.tile([128, T], F32, name=f"qkT{i}", tag=f"qkT{i}")
               for i in range(4)]  # qp0, qp1, kp0, kp1
        v_sb = [persist.tile([128, 4 * 65], F32, name=f"vsb{i}", tag=f"vsb{i}")
                for i in range(TB)]
        attnT = [persist.tile([128, T], F32, name=f"attnT{i}", tag=f"attnT{i}")
                 for i in range(2)]
        wout_sb = [persist.tile([128, E], F32, name=f"wout{i}", tag=f"wout{i}")
                   for i in range(2)]
        for i in range(2):
            nc.sync.dma_start(out=r(wout_sb[i]),
                              in_=r(wout_d[i * 128:(i + 1) * 128, :]))
        # ones column for the fused softmax denominator
        for i in range(TB):
            ones_ap = v_sb[i].rearrange("p (h w) -> p h w", w=65)[:, :, 64:65]
            nc.gpsimd.memset(ones_ap.bitcast(mybir.dt.uint32), 0x3F800000)

        # ---- phase 1: projections (+ rope) ----
        with tc.tile_pool(name="ph1", bufs=1) as ph1, \
             tc.tile_pool(name="ph1ps", bufs=1, space="PSUM") as ph1ps:
            xT = [ph1.tile([128, T], F32, name=f"xT{i}", tag=f"xT{i}")
                  for i in range(KC)]
            wqk = [ph1.tile([128, 512], F32, name=f"wqk{i}", tag=f"wqk{i}")
                   for i in range(KC)]
            wv = [ph1.tile([128, 256], F32, name=f"wv{i}", tag=f"wv{i}")
                  for i in range(KC)]
            for i in range(KC):
                nc.sync.dma_start(out=r(wv[i]),
                                  in_=r(wv_d[i * 128:(i + 1) * 128, :]))
                nc.sync.dma_start(out=r(wqk[i]),
                                  in_=r(wqk_d[i * 128:(i + 1) * 128, :]))
            for half in range(2):
                hs = slice(half * (T // 2), (half + 1) * (T // 2))
                for i in range(KC):
                    # bf16 over the wire, staged + upconverted on the
                    # Pool engine - halves the dominant input DMA
                    xtb = ph1.tile([128, T // 2], BF16, tag="xtb", bufs=2)
                    nc.sync.dma_start(
                        out=xtb,
                        in_=xT_d[i * 128:(i + 1) * 128, hs])
                    nc.gpsimd.tensor_copy(r(xT[i][:, hs]), xtb)
            tabs = {}
            for nm, dd in (("cosk", cosk_d), ("sink", sink_d)):
                tabs[nm] = ph1.tile([128, T], F32, name=nm, tag=nm)
                nc.sync.dma_start(out=tabs[nm], in_=dd[:, :])

            # v in natural [T, D] layout, 4 heads side by side
            for tb in range(TB):
                vps = ph1ps.tile([128, 256], F32, tag="vps", bufs=2)
                for kc in range(KC):
                    nc.tensor.matmul(
                        vps,
                        r(xT[kc][:, tb * 128:(tb + 1) * 128]),
                        r(wv[kc]),
                        start=(kc == 0), stop=(kc == KC - 1),
                    )
                vdst = v_sb[tb].rearrange("p (h w) -> p h w", w=65)[:, :, 0:64]
                nc.vector.tensor_copy(r(vdst), vps)

            # q/k pair tiles, rope fused into the psum->sbuf path
            # DVE: shuffle + first rope multiply; Pool: second multiply + add
            for mb in (0, 1, 2, 3):      # qp0, qp1, kp0, kp1
                ct, st = tabs["cosk"], tabs["sink"]
                for ch in range(NCH):
                    cols = slice(ch * 512, (ch + 1) * 512)
                    qkps = ph1ps.tile([128, 512], F32, tag="qkps", bufs=5)
                    for kc in range(KC):
                        nc.tensor.matmul(
                            qkps,
                            r(wqk[kc][:, mb * 128:(mb + 1) * 128]),
                            r(xT[kc][:, cols]),
                            start=(kc == 0), stop=(kc == KC - 1),
                        )
                    shf = ph1.tile([128, 512], F32, tag="shf", bufs=2)
                    nc.vector.stream_shuffle(shf, qkps, swap_mask)
                    # 1/sqrt(D) is pre-folded into wq on the host (exact:
                    # 0.125 = 2^-3), so both rope multiplies are plain TT
                    # ops and the second one + the add run on Pool
                    t1 = ph1.tile([128, 512], F32, tag="t1", bufs=2)
                    nc.vector.tensor_mul(t1, qkps, ct[:, cols])
                    t2 = ph1.tile([128, 512], F32, tag="t2", bufs=2)
                    nc.gpsimd.tensor_mul(t2, shf, st[:, cols])
                    nc.gpsimd.tensor_add(r(qkT[mb][:, cols]), t1, t2)

        # ---- phase 2: attention ----
        if 2 in phases:
         with tc.tile_pool(name="ph2", bufs=1) as ph2, \
             tc.tile_pool(name="ph2d", bufs=1, space="DRAM") as ph2d, \
             tc.tile_pool(name="ph2ps", bufs=1, space="PSUM") as ph2ps:
            # pre-zeroed diagonal prob buffers: tag ptd{j} keeps cols
            # [0, 128j) of each head half zero forever; exp only ever
            # writes [128j, 512)
            for j in (1, 2, 3):
                for _b in range(2):
                    ptz = ph2.tile([128, 1024], F32, tag=f"ptd{j}", bufs=2)
                    ptz3 = ptz.rearrange("p (h w) -> p h w", h=2)
                    nc.gpsimd.memset(ptz3[:, :, 0:j * 128].bitcast(mybir.dt.uint32), 0)
            for p in range(2):           # head pairs
                qT, kT = qkT[p], qkT[2 + p]
                for ch in range(NCH):
                    qcols = slice(ch * 512, (ch + 1) * 512)
                    nkj = 4 * ch + 4
                    ops = ph2ps.tile([65, 1024], F32, tag="ops", bufs=2)

                    def av(kj, pt):
                        for hh in range(2):
                            h = 2 * p + hh
                            nc.tensor.matmul(
                                ops[:, hh * 512:(hh + 1) * 512],
                                r(v_sb[kj][:, h * 65:(h + 1) * 65]),
                                r(pt[:, hh * 512:(hh + 1) * 512]),
                                start=(kj == 0), stop=(kj == nkj - 1),
                                skip_group_check=True,
                            )

                    pending = []
                    for kj in range(nkj):
                        sps = ph2ps.tile([128, 1024], F32, tag="sps", bufs=2)
                        kcols = slice(kj * 128, (kj + 1) * 128)
                        j = kj - 4 * ch          # >=0 on diagonal blocks
                        s_true = max(0, j) * 128
                        nc.tensor.matmul(
                            sps[:, 0:512],
                            r(kT[0:64, kcols]), r(qT[0:64, qcols]),
                            start=True, stop=True,
                        )
                        nc.tensor.matmul(
                            sps[:, 512:1024],
                            r(kT[64:128, kcols]), r(qT[64:128, qcols]),
                            start=True, stop=True,
                        )
                        if j >= 1:
                            pt = ph2.tile([128, 1024], F32, tag=f"ptd{j}",
                                          bufs=2)
                        else:
                            pt = ph2.tile([128, 1024], F32, tag="pt", bufs=6)
                        sps3 = sps.rearrange("p (h w) -> p h w", h=2)
                        pt3 = pt.rearrange("p (h w) -> p h w", h=2)
                        nc.scalar.activation(
                            r(pt3[:, :, s_true:512]), sps3[:, :, s_true:512],
                            AF.Exp)
                        if j >= 0:
                            # zero where q < k on the 128-wide jagged strip
                            nc.gpsimd.affine_select(
                                out=r(pt3[:, :, s_true:s_true + 128]),
                                in_=r(pt3[:, :, s_true:s_true + 128]),
                                compare_op=ALU.is_ge,
                                fill=0.0,
                                base=0,
                                channel_multiplier=-1,
                                pattern=[[0, 2], [1, 128]],
                            )
                        pending.append((kj, pt))
                        if len(pending) > 5:
                            av(*pending.pop(0))
                    for item in pending:
                        av(*item)
                    # normalize: reciprocal of the denom row straight from
                    # psum, one DRAM bounce for the partition broadcast
                    rcp1 = ph2.tile([1, 1024], F32, tag="rcp1", bufs=2)
                    nc.vector.reciprocal(rcp1, ops[64:65, :])
                    rnd = ph2d.tile([1, 1024], F32, tag="rnd", bufs=2)
                    nc.sync.dma_start(out=rnd[0:1, :], in_=rcp1[0:1, :])
                    bc = ph2.tile([64, 1024], F32, tag="bc", bufs=2)
                    bcast_src = bass.AP(
                        tensor=rnd.tensor, offset=rnd.offset,
                        ap=[[0, 64]] + [list(d) for d in rnd.ap[1:]],
                    )
                    nc.sync.dma_start(out=bc, in_=bcast_src)
                    for hh in range(2):
                        nc.vector.tensor_mul(
                            r(attnT[p][hh * 64:(hh + 1) * 64, qcols]),
                            ops[0:64, hh * 512:(hh + 1) * 512],
                            bc[:, hh * 512:(hh + 1) * 512],
                        )

        # ---- phase 3: output projection ----
        if 3 in phases:
         with tc.tile_pool(name="ph3", bufs=1) as ph3, \
             tc.tile_pool(name="ph3ps", bufs=1, space="PSUM") as ph3ps:
            for qb in range(TB):
                yps = ph3ps.tile([128, E], F32, tag="yps", bufs=2)
                for chunk in range(2):
                    for nh in range(2):
                        nc.tensor.matmul(
                            yps[:, nh * 512:(nh + 1) * 512],
                            r(attnT[chunk][:, qb * 128:(qb + 1) * 128]),
                            r(wout_sb[chunk][:, nh * 512:(nh + 1) * 512]),
                            start=(chunk == 0), stop=(chunk == 1),
                            skip_group_check=True,
                        )
                ysb = ph3.tile([128, E], F32, tag="ysb", bufs=3)
                nc.vector.tensor_copy(ysb, yps)
                nc.sync.dma_start(out=y_d[qb * 128:(qb + 1) * 128, :], in_=ysb)

        persist.release()

    nc.compile()
    return nc


def _host_prep(x, w_qkv, w_out):
    """Build the 8 per-core input maps."""
    inv_freq = 1.0 / (10000.0 ** (np.arange(0, D, 2, dtype=np.float64) / D))
    t = np.arange(T, dtype=np.float64)
    ang = np.outer(inv_freq, t)                    # [32, T]
    cos_f = np.cos(ang)
    sin_f = np.sin(ang)
    cos_t = np.empty((128, T), dtype=np.float32)
    sin_t = np.empty((128, T), dtype=np.float32)
    for rr in range(128):
        f = (rr % 64) // 2
        sgn = -1.0 if (rr % 2 == 0) else 1.0
        cos_t[rr] = cos_f[f]
        sin_t[rr] = sgn * sin_f[f]

    # interleave head-dim pairs (d, d+32) -> rows (2f, 2f+1)
    perm = np.empty(D, dtype=np.int64)
    for f in range(32):
        perm[2 * f] = f
        perm[2 * f + 1] = f + 32

    w_q = w_qkv[:, 0:E]
    w_k = w_qkv[:, E:2 * E]
    w_v = w_qkv[:, 2 * E:3 * E]

    in_maps = []
    for c in range(N_CORES):
        b, hg = divmod(c, HG)
        heads = [4 * hg + i for i in range(4)]
        xT = np.ascontiguousarray(x[b].T)
        wqk = np.concatenate(
            [w_q[:, h * D:(h + 1) * D][:, perm] * np.float32(0.125)
             for h in heads]
            + [w_k[:, h * D:(h + 1) * D][:, perm] for h in heads], axis=1)
        wv = np.concatenate(
            [w_v[:, h * D:(h + 1) * D] for h in heads], axis=1)
        wout = w_out[hg * 256:(hg + 1) * 256, :]
        in_maps.append({
            "xT": np.ascontiguousarray(xT).astype(ml_dtypes.bfloat16),
            "wqk": np.ascontiguousarray(wqk),
            "wv": np.ascontiguousarray(wv),
            "wout": np.ascontiguousarray(wout),
            "cosk": cos_t, "sink": sin_t,
        })
    return in_maps


def kernel(x, w_qkv, w_out):
    x = np.asarray(x, dtype=np.float32)
    w_qkv = np.asarray(w_qkv, dtype=np.float32)
    w_out = np.asarray(w_out, dtype=np.float32)

    if "nc" not in _CACHE:
        _CACHE["nc"] = build_program()
    nc = _CACHE["nc"]

    in_maps = _host_prep(x, w_qkv, w_out)
    res = run_bass_kernel_spmd(nc, in_maps, list(range(N_CORES)))
    _CACHE["last_results"] = res

    y = np.zeros((B, T, E), dtype=np.float32)
    for c in range(N_CORES):
        b = c // HG
        y[b] += res.results[c]["y"]
    return y
